# revision 30
# baseline (speedup 1.0000x reference)
"""Trainium2 Bass kernel for nn_LocallyDense (gather -> 41 grouped GEMMs -> concat
-> Dense -> LeakyReLU), sharded over 8 NeuronCores.

Sharding: expert-parallel over groups. Each core owns 5 full groups (slots 0-4)
plus 1/8 of group 40's contraction dim (slot 5) — legal because the final
Dense is contraction-sharded and the cross-core ReduceScatter sums partial
products, so partial hT contributions for a split group sum correctly by
linearity. This gives every core exactly 10496+pad gathered rows (perfect
balance, no dummy slots) with a single SPMD NEFF.

The gather runs as SWDGE dma_gather over x^T (bf16): the int16 index limit
(D=65536 > 32767) is handled by splitting each slot's indices into lo(<32768)
/ hi(>=32768, rebased) segments, each padded to a global fixed size with dummy
index 0 whose W rows are zeroed; indices are sorted ascending per segment for
HBM locality. Gathers are spread over SWDGE queues 1-3 (async Q7 workers,
~4x desc-gen parallelism; queue 0 would block the engine). Phase-1 and
phase-2 GEMMs run in bf16 (PSUM accumulates fp32). The tail exchanges bf16
partials with an AllToAll, reduces the 8 per-core contributions with a
ones-matrix matmul on the PE (cross-partition sum), then bias+LeakyReLU;
the host concatenates the per-core [16,2,E] shards.
"""

import numpy as np
import ml_dtypes

import concourse.bacc as bacc
import concourse.bass as bass
import concourse.mybir as mybir
import concourse.tile as tile
from concourse.bass_utils import run_bass_kernel_spmd

NCORES = 8
FULL_SLOTS = 5          # full groups per core
SLOTS = FULL_SLOTS + 1  # + 1 split-group slot
B, D, N, G, O, E = 256, 65536, 41, 2048, 256, 512
HALF = 32768
K2 = SLOTS * 2          # hT k-chunks per core
F32 = mybir.dt.float32
BF16 = mybir.dt.bfloat16
I16 = mybir.dt.int16
NEG_SLOPE = 0.2
BF = ml_dtypes.bfloat16


def _pad128(n):
    return -(-n // 128) * 128


def _prep_inputs(x, group_idx, W, b, W3, b3):
    """Host-side sharding/layout prep. Returns (in_maps, sizes dict)."""
    group_idx = group_idx.astype(np.int64)

    # slot assignment: core c -> groups [5c, 5c+5) + group 40 rows [256c, 256c+256)
    SPAN = G // NCORES  # 256
    lo_masks = group_idx < HALF

    S_LO = max(_pad128(int(lo_masks[n].sum())) for n in range(FULL_SLOTS * NCORES))
    S_HI = max(_pad128(G - int(lo_masks[n].sum())) for n in range(FULL_SLOTS * NCORES))
    s6lo = [int(lo_masks[40, c * SPAN : (c + 1) * SPAN].sum()) for c in range(NCORES)]
    S_LO6 = max(_pad128(v) for v in s6lo)
    S_HI6 = max(_pad128(SPAN - v) for v in s6lo)
    C = (S_LO + S_HI) // 128
    C6 = (S_LO6 + S_HI6) // 128

    xTb = np.ascontiguousarray(x.T.astype(BF))  # (D, B) bf16
    b3bc = np.ascontiguousarray(np.broadcast_to(b3, (16, E))).astype(np.float32)

    def idx_pattern(arr, S):
        """(S,) int16 -> [128, S/16] wrapped+replicated pattern."""
        pat = arr.reshape(S // 16, 16).T  # (16, S/16)
        return np.tile(pat, (8, 1))

    def split_pad(idx, S_lo, S_hi):
        """Returns (idx_lo padded, idx_hi padded, lo_positions, hi_positions).

        Positions are sorted by index value so the gather reads ascending
        HBM addresses (better row-buffer locality than random order); the
        W rows are permuted to match, so the GEMM is unaffected."""
        lo_pos = np.where(idx < HALF)[0]
        hi_pos = np.where(idx >= HALF)[0]
        lo_pos = lo_pos[np.argsort(idx[lo_pos], kind="stable")]
        hi_pos = hi_pos[np.argsort(idx[hi_pos], kind="stable")]
        il = np.zeros(S_lo, np.int16)
        ih = np.zeros(S_hi, np.int16)
        il[: len(lo_pos)] = idx[lo_pos].astype(np.int16)
        ih[: len(hi_pos)] = (idx[hi_pos] - HALF).astype(np.int16)
        return il, ih, lo_pos, hi_pos

    in_maps = []
    for core in range(NCORES):
        idx_lo = np.zeros((128, FULL_SLOTS, S_LO // 16), np.int16)
        idx_hi = np.zeros((128, FULL_SLOTS, S_HI // 16), np.int16)
        Wp = np.zeros((FULL_SLOTS, S_LO + S_HI, O), np.float32)
        bias = np.zeros((128, K2), np.float32)
        W3l = np.zeros((K2 * 128, E), np.float32)  # cast to bf16 at layout time
        for s in range(FULL_SLOTS):
            n = core * FULL_SLOTS + s
            il, ih, lo_pos, hi_pos = split_pad(group_idx[n], S_LO, S_HI)
            idx_lo[:, s, :] = idx_pattern(il, S_LO)
            idx_hi[:, s, :] = idx_pattern(ih, S_HI)
            Wp[s, : len(lo_pos)] = W[n, lo_pos]
            Wp[s, S_LO : S_LO + len(hi_pos)] = W[n, hi_pos]
            bias[:, s * 2] = b[n, 0:128]
            bias[:, s * 2 + 1] = b[n, 128:256]
            W3l[s * 256 : (s + 1) * 256] = W3[n * 256 : (n + 1) * 256]
        # slot 5: 1/8 of group 40's contraction dim
        span = group_idx[40, core * SPAN : (core + 1) * SPAN]
        il6, ih6, lo6, hi6 = split_pad(span, S_LO6, S_HI6)
        Wp6 = np.zeros((S_LO6 + S_HI6, O), np.float32)
        Wp6[: len(lo6)] = W[40, core * SPAN + lo6]
        Wp6[S_LO6 : S_LO6 + len(hi6)] = W[40, core * SPAN + hi6]
        if core == 0:
            bias[:, 10] = b[40, 0:128]
            bias[:, 11] = b[40, 128:256]
        W3l[10 * 128 : 12 * 128] = W3[40 * 256 : 41 * 256]

        # device layouts
        Wp_dev = (
            Wp.reshape(FULL_SLOTS, C, 128, O).transpose(0, 2, 1, 3)
            .reshape(FULL_SLOTS, 128, C * O).astype(BF)
        )
        Wp6_dev = (
            Wp6.reshape(C6, 128, O).transpose(1, 0, 2).reshape(128, C6 * O).astype(BF)
        )
        W3_dev = np.ascontiguousarray(
            W3l.reshape(K2, 128, E).transpose(1, 0, 2).reshape(128, K2 * E)
        ).astype(BF)
        pmat = np.zeros((128, 16), np.float32)
        pmat[np.arange(128), np.arange(128) % 16] = 1.0
        pmat = pmat.astype(BF)
        in_maps.append(
            {
                "pmat": pmat,
                "xTb": xTb,
                "idx_lo": np.ascontiguousarray(idx_lo),
                "idx_hi": np.ascontiguousarray(idx_hi),
                "idx_lo6": np.ascontiguousarray(idx_pattern(il6, S_LO6)),
                "idx_hi6": np.ascontiguousarray(idx_pattern(ih6, S_HI6)),
                "Wp": np.ascontiguousarray(Wp_dev),
                "Wp6": np.ascontiguousarray(Wp6_dev),
                "W3l": W3_dev,
                "bias": bias,
                "b3bc": b3bc,
            }
        )
    return in_maps, dict(S_LO=S_LO, S_HI=S_HI, S_LO6=S_LO6, S_HI6=S_HI6, C=C, C6=C6)


def _build(sz):
    S_LO, S_HI, S_LO6, S_HI6, C, C6 = (
        sz["S_LO"], sz["S_HI"], sz["S_LO6"], sz["S_HI6"], sz["C"], sz["C6"]
    )

    nc = bacc.Bacc(num_devices=NCORES, num_swdge_queues=4)
    xT_d = nc.dram_tensor("xTb", [D, B], BF16, kind="ExternalInput")
    il_d = nc.dram_tensor("idx_lo", [128, FULL_SLOTS, S_LO // 16], I16, kind="ExternalInput")
    ih_d = nc.dram_tensor("idx_hi", [128, FULL_SLOTS, S_HI // 16], I16, kind="ExternalInput")
    il6_d = nc.dram_tensor("idx_lo6", [128, S_LO6 // 16], I16, kind="ExternalInput")
    ih6_d = nc.dram_tensor("idx_hi6", [128, S_HI6 // 16], I16, kind="ExternalInput")
    wp_d = nc.dram_tensor("Wp", [FULL_SLOTS, 128, C * O], BF16, kind="ExternalInput")
    wp6_d = nc.dram_tensor("Wp6", [128, C6 * O], BF16, kind="ExternalInput")
    w3_d = nc.dram_tensor("W3l", [128, K2 * E], BF16, kind="ExternalInput")
    bias_d = nc.dram_tensor("bias", [128, K2], F32, kind="ExternalInput")
    b3_d = nc.dram_tensor("b3bc", [16, E], F32, kind="ExternalInput")
    pmat_d = nc.dram_tensor("pmat", [128, 16], BF16, kind="ExternalInput")
    out_d = nc.dram_tensor("out", [16, 2, E], F32, kind="ExternalOutput")

    with tile.TileContext(nc) as tc:
        with (
            tc.tile_pool(name="const", bufs=1) as constp,
            tc.tile_pool(name="gpool", bufs=6) as gpool,
            tc.tile_pool(name="wpool", bufs=6) as wpool,
            tc.tile_pool(name="ps1", bufs=4, space="PSUM") as ps1,
            tc.tile_pool(name="ps2", bufs=1, space="PSUM") as ps2,
            tc.tile_pool(name="psr", bufs=1, space="PSUM") as psr,
            tc.tile_pool(name="dram", bufs=1, space="DRAM") as dramp,
        ):
            il_t = constp.tile([128, FULL_SLOTS, S_LO // 16], I16)
            ih_t = constp.tile([128, FULL_SLOTS, S_HI // 16], I16)
            il6_t = constp.tile([128, S_LO6 // 16], I16)
            ih6_t = constp.tile([128, S_HI6 // 16], I16)
            bias_t = constp.tile([128, K2], F32)
            b3_t = constp.tile([16, E], F32)
            w3_t = constp.tile([128, K2, E], BF16)
            # idx loads ride the second HWDGE ring (Activation) so they are
            # not queued behind the weight streams on sync, and the first
            # gathers can start as early as possible
            nc.scalar.dma_start(il_t[:], il_d[:])
            nc.scalar.dma_start(ih_t[:], ih_d[:])
            nc.scalar.dma_start(il6_t[:], il6_d[:])
            nc.scalar.dma_start(ih6_t[:], ih6_d[:])

            # SDMA round-robins rings at packet granularity, so MB-scale
            # weight streams starve the tiny idx transfers and push the first
            # gather out by ~20us. Gate the big weight DMAs on idx arrival:
            # flag_t reads the idx tiles (RAW dep), and a dummy write into
            # each weight tile (WAW dep) holds its DMA back until then.
            flag_t = constp.tile([1, 16], I16)
            nc.vector.tensor_copy(flag_t[:], il_t[0:1, 0, 0:16])
            nc.vector.tensor_copy(flag_t[:], ih_t[0:1, 0, 0:16])
            nc.vector.tensor_copy(flag_t[:], il6_t[0:1, 0:16])
            nc.vector.tensor_copy(flag_t[:], ih6_t[0:1, 0:16])

            hT_t = constp.tile([128, K2, B], BF16)

            # Gathers ride SWDGE queues 1-3 ONLY: their dispatch retires in
            # ~60ns to async Q7 workers, while a queue-0 gather desc-gens
            # inline on the GpSimd engine (~10.7us) and blocks dispatch of
            # every later gather — observed to delay the final round ~10us.
            # q1/q2 carry the lo/hi halves of s0,s1,s2,s4 (one slot ready per
            # ~10.7us), q3 carries the small s5 first then both s3 halves.
            gather_order = [0, SLOTS - 1, 1, 2, 3, 4]
            # PE consumption order = order of data readiness
            slot_order = [SLOTS - 1, 0, 1, 3, 2, 4]
            queue_for = {
                (0, 0): 1, (0, 1): 2,
                (SLOTS - 1, 0): 3, (SLOTS - 1, 1): 3,
                (1, 0): 1, (1, 1): 2,
                (2, 0): 1, (2, 1): 2,
                (3, 0): 3, (3, 1): 3,
                (4, 0): 1, (4, 1): 2,
            }
            gts = {}
            for s in gather_order:
                cs = C if s < FULL_SLOTS else C6
                gt = gpool.tile([128, cs, B], BF16, tag="gt" if s < FULL_SLOTS else "gt6")
                if s < FULL_SLOTS:
                    nc.gpsimd.dma_gather(
                        gt[:, 0 : S_LO // 128, :], xT_d[0:HALF, :], il_t[:, s, :],
                        S_LO, S_LO, B, single_packet=False, queue_num=queue_for[(s, 0)],
                    )
                    nc.gpsimd.dma_gather(
                        gt[:, S_LO // 128 : cs, :], xT_d[HALF:D, :], ih_t[:, s, :],
                        S_HI, S_HI, B, single_packet=False, queue_num=queue_for[(s, 1)],
                    )
                else:
                    nc.gpsimd.dma_gather(
                        gt[:, 0 : S_LO6 // 128, :], xT_d[0:HALF, :], il6_t[:],
                        S_LO6, S_LO6, B, single_packet=False, queue_num=queue_for[(s, 0)],
                    )
                    nc.gpsimd.dma_gather(
                        gt[:, S_LO6 // 128 : cs, :], xT_d[HALF:D, :], ih6_t[:],
                        S_HI6, S_HI6, B, single_packet=False, queue_num=queue_for[(s, 1)],
                    )
                wt = wpool.tile([128, cs, O], BF16, tag="wt" if s < FULL_SLOTS else "wt6")
                gts[s] = (gt, wt, cs)

            # Weight streams, sync ring (FIFO). The first ~3MB (s0+s5 wt, W3,
            # small consts) go UNGATED — they drain in the otherwise-dead SDMA
            # window before the first gather descriptors arrive. The remaining
            # 4.7MB (s1-s4 wt) are gated behind idx arrival via flag_t so they
            # cannot starve the tiny idx DMAs at startup.
            def emit_wt_dma(s, gated):
                gt, wt, cs = gts[s]
                if gated:
                    nc.vector.tensor_copy(wt[0:1, 0, 0:16], flag_t[:])
                if s < FULL_SLOTS:
                    nc.sync.dma_start(wt[:], wp_d[s].rearrange("p (c o) -> p c o", o=O))
                else:
                    nc.sync.dma_start(wt[:], wp6_d[:].rearrange("p (c o) -> p c o", o=O))

            emit_wt_dma(0, gated=False)
            emit_wt_dma(SLOTS - 1, gated=False)
            nc.sync.dma_start(w3_t[:], w3_d[:].rearrange("p (k e) -> p k e", e=E))
            nc.sync.dma_start(bias_t[:], bias_d[:])
            nc.sync.dma_start(b3_t[:], b3_d[:])
            pmat_t = constp.tile([128, 16], BF16)
            nc.sync.dma_start(pmat_t[:], pmat_d[:])
            for s in (1, 2, 3, 4):
                emit_wt_dma(s, gated=True)

            # phase-2 PSUM banks accumulate across the whole slot loop, so the
            # final Dense adds no PE tail after the last slot's phase-1 GEMM
            p2_0 = ps2.tile([128, E], F32, tag="p2_0")
            p2_1 = ps2.tile([128, E], F32, tag="p2_1")
            p2 = [p2_0, p2_1]

            def emit_phase2(si, s):
                for bh in range(2):
                    for oh in range(2):
                        kc = s * 2 + oh
                        nc.tensor.matmul(
                            p2[bh][:],
                            hT_t[:, kc, bh * 128 : (bh + 1) * 128],
                            w3_t[:, kc, :],
                            start=(si == 0 and oh == 0),
                            stop=(si == len(slot_order) - 1 and oh == 1),
                        )

            # phase-2 for slot k is emitted during slot k+1's phase-1 so the
            # PE never waits on the DVE bias-add round trip
            for si, s in enumerate(slot_order):
                gt, wt, cs = gts[s]
                for oh in range(2):
                    ps = ps1.tile([128, B], F32)
                    for cc in range(cs):
                        nc.tensor.matmul(
                            ps[:],
                            wt[:, cc, oh * 128 : (oh + 1) * 128],
                            gt[:, cc, :],
                            start=(cc == 0),
                            stop=(cc == cs - 1),
                        )
                    kc = s * 2 + oh
                    nc.vector.tensor_scalar_add(
                        hT_t[:, kc, :], ps[:], bias_t[:, kc : kc + 1]
                    )
                if si > 0:
                    emit_phase2(si - 1, slot_order[si - 1])
            emit_phase2(len(slot_order) - 1, slot_order[-1])

            # Tail: AllToAll (floor ~4.7us vs ReduceScatter ~7.3us, no CCE on
            # the wire) exchanges bf16 partials, then one PE matmul against
            # pmat (ones at (i, i%16)) sums the 8 per-core contributions that
            # land on partition groups 16j..16j+16 — a cross-partition reduce
            # the DVE cannot do. Partials in bf16 add ~0.1% error (budget 2e-2).
            part_t = constp.tile([128, 2, E], BF16)
            for bh in range(2):
                nc.vector.tensor_copy(part_t[:, bh, :], p2[bh][:])

            ccin = dramp.tile([128, 2, E], BF16)
            ccout = dramp.tile([128, 2, E], BF16)
            nc.sync.dma_start(ccin[:], part_t[:])
            nc.gpsimd.collective_compute(
                "AllToAll",
                mybir.AluOpType.bypass,
                replica_groups=[list(range(NCORES))],
                ins=[ccin[:].opt()],
                outs=[ccout[:].opt()],
            )
            a2a_t = constp.tile([128, 2, E], BF16)
            nc.sync.dma_start(a2a_t[:], ccout[:])
            pr = [
                psr.tile([16, E], F32, tag=f"pr{bh}", name=f"pr{bh}")
                for bh in range(2)
            ]
            for bh in range(2):
                nc.tensor.matmul(
                    pr[bh][:], pmat_t[:], a2a_t[:, bh, :], start=True, stop=True,
                )
            z_t = constp.tile([16, 2, E], F32)
            for bh in range(2):
                nc.vector.tensor_add(z_t[:, bh, :], pr[bh][:], b3_t[:])
            o_t = constp.tile([16, 2, E], F32)
            # LeakyReLU: max(0.2*z, z)
            nc.vector.scalar_tensor_tensor(
                o_t[:], z_t[:], NEG_SLOPE, z_t[:],
                op0=mybir.AluOpType.mult, op1=mybir.AluOpType.max,
            )
            nc.sync.dma_start(out_d[:], o_t[:])
    nc.compile()
    return nc


def kernel_with_results(x, group_idx, W, b, W3, b3, trace=False, warmup=True):
    in_maps, sz = _prep_inputs(
        np.asarray(x, dtype=np.float32),
        np.asarray(group_idx),
        np.asarray(W, dtype=np.float32),
        np.asarray(b, dtype=np.float32),
        np.asarray(W3, dtype=np.float32),
        np.asarray(b3, dtype=np.float32),
    )
    nc = _build(sz)
    if warmup:
        # first execute pays NEFF-load / runtime-init cross-core skew; the
        # measured run below then starts with all 8 cores aligned
        run_bass_kernel_spmd(nc, in_maps, core_ids=list(range(NCORES)))
    import os as _os

    extra = int(_os.environ.get("N_RUNS", "1")) - 1
    for _ in range(extra):
        r = run_bass_kernel_spmd(
            nc, in_maps, core_ids=list(range(NCORES)), trace=trace
        )
        if r.exec_time_ns is not None:
            print(f"run exec: {r.exec_time_ns} ns")
    res = run_bass_kernel_spmd(
        nc, in_maps, core_ids=list(range(NCORES)), trace=trace
    )
    out = np.empty((B, E), np.float32)
    for c in range(NCORES):
        shard = res.results[c]["out"]  # (16, 2, E): rows 16c..16c+16 of each b-half
        out[16 * c : 16 * c + 16, :] = shard[:, 0, :]
        out[128 + 16 * c : 128 + 16 * c + 16, :] = shard[:, 1, :]
    return out, res


def kernel(**inputs):
    out, _ = kernel_with_results(**inputs)
    return out



# revision 31
# speedup vs baseline: 1.0366x; 1.0366x over previous
"""Trainium2 Bass kernel for nn_LocallyDense (gather -> 41 grouped GEMMs -> concat
-> Dense -> LeakyReLU), sharded over 8 NeuronCores.

Sharding: expert-parallel over groups. Each core owns 5 full groups (slots 0-4)
plus 1/8 of group 40's contraction dim (slot 5) — legal because the final
Dense is contraction-sharded and the cross-core ReduceScatter sums partial
products, so partial hT contributions for a split group sum correctly by
linearity. This gives every core exactly 10496+pad gathered rows (perfect
balance, no dummy slots) with a single SPMD NEFF.

The gather runs as SWDGE dma_gather over x^T (bf16): the int16 index limit
(D=65536 > 32767) is handled by splitting each slot's indices into lo(<32768)
/ hi(>=32768, rebased) segments, each padded to a global fixed size with dummy
index 0 whose W rows are zeroed; indices are sorted ascending per segment for
HBM locality. Gathers are spread over SWDGE queues 1-3 (async Q7 workers,
~4x desc-gen parallelism; queue 0 would block the engine). Phase-1 and
phase-2 GEMMs run in bf16 (PSUM accumulates fp32). The tail exchanges bf16
partials with an AllToAll, reduces the 8 per-core contributions with a
ones-matrix matmul on the PE (cross-partition sum), then bias+LeakyReLU;
the host concatenates the per-core [16,2,E] shards.
"""

import numpy as np
import ml_dtypes

import concourse.bacc as bacc
import concourse.bass as bass
import concourse.mybir as mybir
import concourse.tile as tile
from concourse.bass_utils import run_bass_kernel_spmd

NCORES = 8
FULL_SLOTS = 5          # full groups per core
SLOTS = FULL_SLOTS + 1  # + 1 split-group slot
B, D, N, G, O, E = 256, 65536, 41, 2048, 256, 512
HALF = 32768
K2 = SLOTS * 2          # hT k-chunks per core
F32 = mybir.dt.float32
BF16 = mybir.dt.bfloat16
I16 = mybir.dt.int16
NEG_SLOPE = 0.2
BF = ml_dtypes.bfloat16


def _pad128(n):
    return -(-n // 128) * 128


def _prep_inputs(x, group_idx, W, b, W3, b3):
    """Host-side sharding/layout prep. Returns (in_maps, sizes dict)."""
    group_idx = group_idx.astype(np.int64)

    # slot assignment: core c -> groups [5c, 5c+5) + group 40 rows [256c, 256c+256)
    SPAN = G // NCORES  # 256
    lo_masks = group_idx < HALF

    S_LO = max(_pad128(int(lo_masks[n].sum())) for n in range(FULL_SLOTS * NCORES))
    S_HI = max(_pad128(G - int(lo_masks[n].sum())) for n in range(FULL_SLOTS * NCORES))
    s6lo = [int(lo_masks[40, c * SPAN : (c + 1) * SPAN].sum()) for c in range(NCORES)]
    S_LO6 = max(_pad128(v) for v in s6lo)
    S_HI6 = max(_pad128(SPAN - v) for v in s6lo)
    C = (S_LO + S_HI) // 128
    C6 = (S_LO6 + S_HI6) // 128

    xTb = np.ascontiguousarray(x.T.astype(BF))  # (D, B) bf16
    b3bc = np.ascontiguousarray(np.broadcast_to(b3, (16, E))).astype(np.float32)

    def idx_pattern(arr, S):
        """(S,) int16 -> [128, S/16] wrapped+replicated pattern."""
        pat = arr.reshape(S // 16, 16).T  # (16, S/16)
        return np.tile(pat, (8, 1))

    def split_pad(idx, S_lo, S_hi):
        """Returns (idx_lo padded, idx_hi padded, lo_positions, hi_positions).

        Positions are sorted by index value so the gather reads ascending
        HBM addresses (better row-buffer locality than random order); the
        W rows are permuted to match, so the GEMM is unaffected."""
        lo_pos = np.where(idx < HALF)[0]
        hi_pos = np.where(idx >= HALF)[0]
        lo_pos = lo_pos[np.argsort(idx[lo_pos], kind="stable")]
        hi_pos = hi_pos[np.argsort(idx[hi_pos], kind="stable")]
        il = np.zeros(S_lo, np.int16)
        ih = np.zeros(S_hi, np.int16)
        il[: len(lo_pos)] = idx[lo_pos].astype(np.int16)
        ih[: len(hi_pos)] = (idx[hi_pos] - HALF).astype(np.int16)
        return il, ih, lo_pos, hi_pos

    in_maps = []
    for core in range(NCORES):
        idx_lo = np.zeros((128, FULL_SLOTS, S_LO // 16), np.int16)
        idx_hi = np.zeros((128, FULL_SLOTS, S_HI // 16), np.int16)
        Wp = np.zeros((FULL_SLOTS, S_LO + S_HI, O), np.float32)
        bias = np.zeros((128, K2), np.float32)
        W3l = np.zeros((K2 * 128, E), np.float32)  # cast to bf16 at layout time
        for s in range(FULL_SLOTS):
            n = core * FULL_SLOTS + s
            il, ih, lo_pos, hi_pos = split_pad(group_idx[n], S_LO, S_HI)
            idx_lo[:, s, :] = idx_pattern(il, S_LO)
            idx_hi[:, s, :] = idx_pattern(ih, S_HI)
            Wp[s, : len(lo_pos)] = W[n, lo_pos]
            Wp[s, S_LO : S_LO + len(hi_pos)] = W[n, hi_pos]
            bias[:, s * 2] = b[n, 0:128]
            bias[:, s * 2 + 1] = b[n, 128:256]
            W3l[s * 256 : (s + 1) * 256] = W3[n * 256 : (n + 1) * 256]
        # slot 5: 1/8 of group 40's contraction dim
        span = group_idx[40, core * SPAN : (core + 1) * SPAN]
        il6, ih6, lo6, hi6 = split_pad(span, S_LO6, S_HI6)
        Wp6 = np.zeros((S_LO6 + S_HI6, O), np.float32)
        Wp6[: len(lo6)] = W[40, core * SPAN + lo6]
        Wp6[S_LO6 : S_LO6 + len(hi6)] = W[40, core * SPAN + hi6]
        if core == 0:
            bias[:, 10] = b[40, 0:128]
            bias[:, 11] = b[40, 128:256]
        W3l[10 * 128 : 12 * 128] = W3[40 * 256 : 41 * 256]

        # device layouts
        Wp_dev = (
            Wp.reshape(FULL_SLOTS, C, 128, O).transpose(0, 2, 1, 3)
            .reshape(FULL_SLOTS, 128, C * O).astype(BF)
        )
        Wp6_dev = (
            Wp6.reshape(C6, 128, O).transpose(1, 0, 2).reshape(128, C6 * O).astype(BF)
        )
        W3_dev = np.ascontiguousarray(
            W3l.reshape(K2, 128, E).transpose(1, 0, 2).reshape(128, K2 * E)
        ).astype(BF)
        pmat = np.zeros((128, 16), np.float32)
        pmat[np.arange(128), np.arange(128) % 16] = 1.0
        pmat = pmat.astype(BF)
        in_maps.append(
            {
                "pmat": pmat,
                "xTb": xTb,
                "idx_lo": np.ascontiguousarray(idx_lo),
                "idx_hi": np.ascontiguousarray(idx_hi),
                "idx_lo6": np.ascontiguousarray(idx_pattern(il6, S_LO6)),
                "idx_hi6": np.ascontiguousarray(idx_pattern(ih6, S_HI6)),
                "Wp": np.ascontiguousarray(Wp_dev),
                "Wp6": np.ascontiguousarray(Wp6_dev),
                "W3l": W3_dev,
                "bias": bias,
                "b3bc": b3bc,
            }
        )
    return in_maps, dict(S_LO=S_LO, S_HI=S_HI, S_LO6=S_LO6, S_HI6=S_HI6, C=C, C6=C6)


def _build(sz):
    S_LO, S_HI, S_LO6, S_HI6, C, C6 = (
        sz["S_LO"], sz["S_HI"], sz["S_LO6"], sz["S_HI6"], sz["C"], sz["C6"]
    )

    nc = bacc.Bacc(num_devices=NCORES, num_swdge_queues=4)
    xT_d = nc.dram_tensor("xTb", [D, B], BF16, kind="ExternalInput")
    il_d = nc.dram_tensor("idx_lo", [128, FULL_SLOTS, S_LO // 16], I16, kind="ExternalInput")
    ih_d = nc.dram_tensor("idx_hi", [128, FULL_SLOTS, S_HI // 16], I16, kind="ExternalInput")
    il6_d = nc.dram_tensor("idx_lo6", [128, S_LO6 // 16], I16, kind="ExternalInput")
    ih6_d = nc.dram_tensor("idx_hi6", [128, S_HI6 // 16], I16, kind="ExternalInput")
    wp_d = nc.dram_tensor("Wp", [FULL_SLOTS, 128, C * O], BF16, kind="ExternalInput")
    wp6_d = nc.dram_tensor("Wp6", [128, C6 * O], BF16, kind="ExternalInput")
    w3_d = nc.dram_tensor("W3l", [128, K2 * E], BF16, kind="ExternalInput")
    bias_d = nc.dram_tensor("bias", [128, K2], F32, kind="ExternalInput")
    b3_d = nc.dram_tensor("b3bc", [16, E], F32, kind="ExternalInput")
    pmat_d = nc.dram_tensor("pmat", [128, 16], BF16, kind="ExternalInput")
    out_d = nc.dram_tensor("out", [16, 2, E], F32, kind="ExternalOutput")

    with tile.TileContext(nc) as tc:
        with (
            tc.tile_pool(name="const", bufs=1) as constp,
            tc.tile_pool(name="gpool", bufs=6) as gpool,
            tc.tile_pool(name="wpool", bufs=6) as wpool,
            tc.tile_pool(name="ps1", bufs=4, space="PSUM") as ps1,
            tc.tile_pool(name="ps2", bufs=1, space="PSUM") as ps2,
            tc.tile_pool(name="psr", bufs=1, space="PSUM") as psr,
            tc.tile_pool(name="dram", bufs=1, space="DRAM") as dramp,
        ):
            il_t = constp.tile([128, FULL_SLOTS, S_LO // 16], I16)
            ih_t = constp.tile([128, FULL_SLOTS, S_HI // 16], I16)
            il6_t = constp.tile([128, S_LO6 // 16], I16)
            ih6_t = constp.tile([128, S_HI6 // 16], I16)
            bias_t = constp.tile([128, K2], F32)
            b3_t = constp.tile([16, E], F32)
            w3_t = constp.tile([128, K2, E], BF16)
            # idx loads ride the second HWDGE ring (Activation) so they are
            # not queued behind the weight streams on sync, and the first
            # gathers can start as early as possible
            nc.scalar.dma_start(il_t[:], il_d[:])
            nc.scalar.dma_start(ih_t[:], ih_d[:])
            nc.scalar.dma_start(il6_t[:], il6_d[:])
            nc.scalar.dma_start(ih6_t[:], ih6_d[:])

            # SDMA round-robins rings at packet granularity, so MB-scale
            # weight streams starve the tiny idx transfers and push the first
            # gather out by ~20us. Gate the big weight DMAs on idx arrival:
            # flag_t reads the idx tiles (RAW dep), and a dummy write into
            # each weight tile (WAW dep) holds its DMA back until then.
            flag_t = constp.tile([1, 16], I16)
            nc.vector.tensor_copy(flag_t[:], il_t[0:1, 0, 0:16])
            nc.vector.tensor_copy(flag_t[:], ih_t[0:1, 0, 0:16])
            nc.vector.tensor_copy(flag_t[:], il6_t[0:1, 0:16])
            nc.vector.tensor_copy(flag_t[:], ih6_t[0:1, 0:16])

            hT_t = constp.tile([128, K2, B], BF16)

            # Gathers spread over the 4 SWDGE queues. Queue 0's desc-gen runs
            # inline on the GpSimd engine (blocks ~10.7us per full gather);
            # queues 1-3 dispatch in ~60ns to async Q7 workers that crunch
            # concurrently. Emission order + queue choice below gets slot 0
            # onto workers immediately, the small slot 5 early, and paces the
            # rest so slots complete roughly every 10.7us.
            gather_order = [0, SLOTS - 1, 1, 2, 3, 4]
            # PE consumption order = order of data readiness
            slot_order = [SLOTS - 1, 0, 1, 2, 3, 4]
            queue_for = {
                (0, 0): 1, (0, 1): 2,
                (SLOTS - 1, 0): 3, (SLOTS - 1, 1): 3,
                (1, 0): 0, (1, 1): 1,
                (2, 0): 2, (2, 1): 3,
                (3, 0): 0, (3, 1): 1,
                (4, 0): 2, (4, 1): 3,
            }
            gts = {}
            for s in gather_order:
                cs = C if s < FULL_SLOTS else C6
                gt = gpool.tile([128, cs, B], BF16, tag="gt" if s < FULL_SLOTS else "gt6")
                if s < FULL_SLOTS:
                    nc.gpsimd.dma_gather(
                        gt[:, 0 : S_LO // 128, :], xT_d[0:HALF, :], il_t[:, s, :],
                        S_LO, S_LO, B, single_packet=False, queue_num=queue_for[(s, 0)],
                    )
                    nc.gpsimd.dma_gather(
                        gt[:, S_LO // 128 : cs, :], xT_d[HALF:D, :], ih_t[:, s, :],
                        S_HI, S_HI, B, single_packet=False, queue_num=queue_for[(s, 1)],
                    )
                else:
                    nc.gpsimd.dma_gather(
                        gt[:, 0 : S_LO6 // 128, :], xT_d[0:HALF, :], il6_t[:],
                        S_LO6, S_LO6, B, single_packet=False, queue_num=queue_for[(s, 0)],
                    )
                    nc.gpsimd.dma_gather(
                        gt[:, S_LO6 // 128 : cs, :], xT_d[HALF:D, :], ih6_t[:],
                        S_HI6, S_HI6, B, single_packet=False, queue_num=queue_for[(s, 1)],
                    )
                wt = wpool.tile([128, cs, O], BF16, tag="wt" if s < FULL_SLOTS else "wt6")
                gts[s] = (gt, wt, cs)

            # Weight streams, sync ring (FIFO). The first ~3MB (s0+s5 wt, W3,
            # small consts) go UNGATED — they drain in the otherwise-dead SDMA
            # window before the first gather descriptors arrive. The remaining
            # 4.7MB (s1-s4 wt) are gated behind idx arrival via flag_t so they
            # cannot starve the tiny idx DMAs at startup.
            def emit_wt_dma(s, gated):
                gt, wt, cs = gts[s]
                if gated:
                    nc.vector.tensor_copy(wt[0:1, 0, 0:16], flag_t[:])
                if s < FULL_SLOTS:
                    nc.sync.dma_start(wt[:], wp_d[s].rearrange("p (c o) -> p c o", o=O))
                else:
                    nc.sync.dma_start(wt[:], wp6_d[:].rearrange("p (c o) -> p c o", o=O))

            emit_wt_dma(0, gated=False)
            emit_wt_dma(SLOTS - 1, gated=False)
            nc.sync.dma_start(w3_t[:], w3_d[:].rearrange("p (k e) -> p k e", e=E))
            nc.sync.dma_start(bias_t[:], bias_d[:])
            nc.sync.dma_start(b3_t[:], b3_d[:])
            pmat_t = constp.tile([128, 16], BF16)
            nc.sync.dma_start(pmat_t[:], pmat_d[:])
            for s in (1, 2, 3, 4):
                emit_wt_dma(s, gated=True)

            # phase-2 PSUM banks accumulate across the whole slot loop, so the
            # final Dense adds no PE tail after the last slot's phase-1 GEMM
            p2_0 = ps2.tile([128, E], F32, tag="p2_0")
            p2_1 = ps2.tile([128, E], F32, tag="p2_1")
            p2 = [p2_0, p2_1]

            def emit_phase2(si, s):
                for bh in range(2):
                    for oh in range(2):
                        kc = s * 2 + oh
                        nc.tensor.matmul(
                            p2[bh][:],
                            hT_t[:, kc, bh * 128 : (bh + 1) * 128],
                            w3_t[:, kc, :],
                            start=(si == 0 and oh == 0),
                            stop=(si == len(slot_order) - 1 and oh == 1),
                        )

            # phase-2 for slot k is emitted during slot k+1's phase-1 so the
            # PE never waits on the DVE bias-add round trip
            for si, s in enumerate(slot_order):
                gt, wt, cs = gts[s]
                for oh in range(2):
                    ps = ps1.tile([128, B], F32)
                    for cc in range(cs):
                        nc.tensor.matmul(
                            ps[:],
                            wt[:, cc, oh * 128 : (oh + 1) * 128],
                            gt[:, cc, :],
                            start=(cc == 0),
                            stop=(cc == cs - 1),
                        )
                    kc = s * 2 + oh
                    nc.vector.tensor_scalar_add(
                        hT_t[:, kc, :], ps[:], bias_t[:, kc : kc + 1]
                    )
                if si > 0:
                    emit_phase2(si - 1, slot_order[si - 1])
            emit_phase2(len(slot_order) - 1, slot_order[-1])

            # Tail: AllToAll (floor ~4.7us vs ReduceScatter ~7.3us, no CCE on
            # the wire) exchanges bf16 partials, then one PE matmul against
            # pmat (ones at (i, i%16)) sums the 8 per-core contributions that
            # land on partition groups 16j..16j+16 — a cross-partition reduce
            # the DVE cannot do. Partials in bf16 add ~0.1% error (budget 2e-2).
            part_t = constp.tile([128, 2, E], BF16)
            for bh in range(2):
                nc.vector.tensor_copy(part_t[:, bh, :], p2[bh][:])

            ccin = dramp.tile([128, 2, E], BF16)
            ccout = dramp.tile([128, 2, E], BF16)
            nc.sync.dma_start(ccin[:], part_t[:])
            nc.gpsimd.collective_compute(
                "AllToAll",
                mybir.AluOpType.bypass,
                replica_groups=[list(range(NCORES))],
                ins=[ccin[:].opt()],
                outs=[ccout[:].opt()],
            )
            a2a_t = constp.tile([128, 2, E], BF16)
            nc.sync.dma_start(a2a_t[:], ccout[:])
            pr = [
                psr.tile([16, E], F32, tag=f"pr{bh}", name=f"pr{bh}")
                for bh in range(2)
            ]
            for bh in range(2):
                nc.tensor.matmul(
                    pr[bh][:], pmat_t[:], a2a_t[:, bh, :], start=True, stop=True,
                )
            z_t = constp.tile([16, 2, E], F32)
            for bh in range(2):
                nc.vector.tensor_add(z_t[:, bh, :], pr[bh][:], b3_t[:])
            o_t = constp.tile([16, 2, E], F32)
            # LeakyReLU: max(0.2*z, z)
            nc.vector.scalar_tensor_tensor(
                o_t[:], z_t[:], NEG_SLOPE, z_t[:],
                op0=mybir.AluOpType.mult, op1=mybir.AluOpType.max,
            )
            nc.sync.dma_start(out_d[:], o_t[:])
    nc.compile()
    return nc


def kernel_with_results(x, group_idx, W, b, W3, b3, trace=False, warmup=True):
    in_maps, sz = _prep_inputs(
        np.asarray(x, dtype=np.float32),
        np.asarray(group_idx),
        np.asarray(W, dtype=np.float32),
        np.asarray(b, dtype=np.float32),
        np.asarray(W3, dtype=np.float32),
        np.asarray(b3, dtype=np.float32),
    )
    nc = _build(sz)
    if warmup:
        # first execute pays NEFF-load / runtime-init cross-core skew; the
        # measured run below then starts with all 8 cores aligned
        run_bass_kernel_spmd(nc, in_maps, core_ids=list(range(NCORES)))
    import os as _os

    extra = int(_os.environ.get("N_RUNS", "1")) - 1
    for _ in range(extra):
        r = run_bass_kernel_spmd(
            nc, in_maps, core_ids=list(range(NCORES)), trace=trace
        )
        if r.exec_time_ns is not None:
            print(f"run exec: {r.exec_time_ns} ns")
    res = run_bass_kernel_spmd(
        nc, in_maps, core_ids=list(range(NCORES)), trace=trace
    )
    out = np.empty((B, E), np.float32)
    for c in range(NCORES):
        shard = res.results[c]["out"]  # (16, 2, E): rows 16c..16c+16 of each b-half
        out[16 * c : 16 * c + 16, :] = shard[:, 0, :]
        out[128 + 16 * c : 128 + 16 * c + 16, :] = shard[:, 1, :]
    return out, res


def kernel(**inputs):
    out, _ = kernel_with_results(**inputs)
    return out



# revision 32
# speedup vs baseline: 1.1856x; 1.1437x over previous
"""Trainium2 Bass kernel for nn_LocallyDense (gather -> 41 grouped GEMMs -> concat
-> Dense -> LeakyReLU), sharded over 8 NeuronCores.

Sharding: expert-parallel over groups. Each core owns 5 full groups (slots 0-4)
plus 1/8 of group 40's contraction dim (slot 5) — legal because the final
Dense is contraction-sharded and the cross-core ReduceScatter sums partial
products, so partial hT contributions for a split group sum correctly by
linearity. This gives every core exactly 10496+pad gathered rows (perfect
balance, no dummy slots) with a single SPMD NEFF.

The gather runs as SWDGE dma_gather over x^T (bf16): the int16 index limit
(D=65536 > 32767) is handled by splitting each slot's indices into lo(<32768)
/ hi(>=32768, rebased) segments, each padded to a global fixed size with dummy
index 0 whose W rows are zeroed; indices are sorted ascending per segment for
HBM locality. Gathers are spread over SWDGE queues 1-3 (async Q7 workers,
~4x desc-gen parallelism; queue 0 would block the engine). Phase-1 and
phase-2 GEMMs run in bf16 (PSUM accumulates fp32). The tail exchanges bf16
partials with an AllToAll, reduces the 8 per-core contributions with a
ones-matrix matmul on the PE (cross-partition sum), then bias+LeakyReLU;
the host concatenates the per-core [16,2,E] shards.
"""

import numpy as np
import ml_dtypes

import concourse.bacc as bacc
import concourse.bass as bass
import concourse.mybir as mybir
import concourse.tile as tile
from concourse.bass_utils import run_bass_kernel_spmd

NCORES = 8
FULL_SLOTS = 5          # full groups per core
SLOTS = FULL_SLOTS + 1  # + 1 split-group slot
B, D, N, G, O, E = 256, 65536, 41, 2048, 256, 512
HALF = 32768
K2 = SLOTS * 2          # hT k-chunks per core
F32 = mybir.dt.float32
BF16 = mybir.dt.bfloat16
I16 = mybir.dt.int16
NEG_SLOPE = 0.2
BF = ml_dtypes.bfloat16


def _pad128(n):
    return -(-n // 128) * 128


def _prep_inputs(x, group_idx, W, b, W3, b3):
    """Host-side sharding/layout prep. Returns (in_maps, sizes dict)."""
    group_idx = group_idx.astype(np.int64)

    # slot assignment: core c -> groups [5c, 5c+5) + group 40 rows [256c, 256c+256)
    SPAN = G // NCORES  # 256
    lo_masks = group_idx < HALF

    S_LO = max(_pad128(int(lo_masks[n].sum())) for n in range(FULL_SLOTS * NCORES))
    S_HI = max(_pad128(G - int(lo_masks[n].sum())) for n in range(FULL_SLOTS * NCORES))
    s6lo = [int(lo_masks[40, c * SPAN : (c + 1) * SPAN].sum()) for c in range(NCORES)]
    S_LO6 = max(_pad128(v) for v in s6lo)
    S_HI6 = max(_pad128(SPAN - v) for v in s6lo)
    C = (S_LO + S_HI) // 128
    C6 = (S_LO6 + S_HI6) // 128

    xTb = np.ascontiguousarray(x.T.astype(BF))  # (D, B) bf16
    b3bc = np.ascontiguousarray(np.broadcast_to(b3, (16, E))).astype(np.float32)

    def idx_pattern(arr, S):
        """(S,) int16 -> [128, S/16] wrapped+replicated pattern."""
        pat = arr.reshape(S // 16, 16).T  # (16, S/16)
        return np.tile(pat, (8, 1))

    def split_pad(idx, S_lo, S_hi):
        """Returns (idx_lo padded, idx_hi padded, lo_positions, hi_positions).

        Positions are sorted by index value so the gather reads ascending
        HBM addresses (better row-buffer locality than random order); the
        W rows are permuted to match, so the GEMM is unaffected."""
        lo_pos = np.where(idx < HALF)[0]
        hi_pos = np.where(idx >= HALF)[0]
        lo_pos = lo_pos[np.argsort(idx[lo_pos], kind="stable")]
        hi_pos = hi_pos[np.argsort(idx[hi_pos], kind="stable")]
        il = np.zeros(S_lo, np.int16)
        ih = np.zeros(S_hi, np.int16)
        il[: len(lo_pos)] = idx[lo_pos].astype(np.int16)
        ih[: len(hi_pos)] = (idx[hi_pos] - HALF).astype(np.int16)
        return il, ih, lo_pos, hi_pos

    in_maps = []
    for core in range(NCORES):
        idx_lo = np.zeros((128, FULL_SLOTS, S_LO // 16), np.int16)
        idx_hi = np.zeros((128, FULL_SLOTS, S_HI // 16), np.int16)
        Wp = np.zeros((FULL_SLOTS, S_LO + S_HI, O), np.float32)
        bias = np.zeros((128, K2), np.float32)
        W3l = np.zeros((K2 * 128, E), np.float32)  # cast to bf16 at layout time
        for s in range(FULL_SLOTS):
            n = core * FULL_SLOTS + s
            il, ih, lo_pos, hi_pos = split_pad(group_idx[n], S_LO, S_HI)
            idx_lo[:, s, :] = idx_pattern(il, S_LO)
            idx_hi[:, s, :] = idx_pattern(ih, S_HI)
            Wp[s, : len(lo_pos)] = W[n, lo_pos]
            Wp[s, S_LO : S_LO + len(hi_pos)] = W[n, hi_pos]
            bias[:, s * 2] = b[n, 0:128]
            bias[:, s * 2 + 1] = b[n, 128:256]
            W3l[s * 256 : (s + 1) * 256] = W3[n * 256 : (n + 1) * 256]
        # slot 5: 1/8 of group 40's contraction dim
        span = group_idx[40, core * SPAN : (core + 1) * SPAN]
        il6, ih6, lo6, hi6 = split_pad(span, S_LO6, S_HI6)
        Wp6 = np.zeros((S_LO6 + S_HI6, O), np.float32)
        Wp6[: len(lo6)] = W[40, core * SPAN + lo6]
        Wp6[S_LO6 : S_LO6 + len(hi6)] = W[40, core * SPAN + hi6]
        if core == 0:
            bias[:, 10] = b[40, 0:128]
            bias[:, 11] = b[40, 128:256]
        W3l[10 * 128 : 12 * 128] = W3[40 * 256 : 41 * 256]

        # device layouts
        Wp_dev = (
            Wp.reshape(FULL_SLOTS, C, 128, O).transpose(0, 2, 1, 3)
            .reshape(FULL_SLOTS, 128, C * O).astype(BF)
        )
        Wp6_dev = (
            Wp6.reshape(C6, 128, O).transpose(1, 0, 2).reshape(128, C6 * O).astype(BF)
        )
        W3_dev = np.ascontiguousarray(
            W3l.reshape(K2, 128, E).transpose(1, 0, 2).reshape(128, K2 * E)
        ).astype(BF)
        pmat = np.zeros((128, 16), np.float32)
        pmat[np.arange(128), np.arange(128) % 16] = 1.0
        pmat = pmat.astype(BF)
        in_maps.append(
            {
                "pmat": pmat,
                "xTb": xTb,
                "idx_lo": np.ascontiguousarray(idx_lo),
                "idx_hi": np.ascontiguousarray(idx_hi),
                "idx_lo6": np.ascontiguousarray(idx_pattern(il6, S_LO6)),
                "idx_hi6": np.ascontiguousarray(idx_pattern(ih6, S_HI6)),
                "Wp": np.ascontiguousarray(Wp_dev),
                "Wp6": np.ascontiguousarray(Wp6_dev),
                "W3l": W3_dev,
                "bias": bias,
                "b3bc": b3bc,
            }
        )
    return in_maps, dict(S_LO=S_LO, S_HI=S_HI, S_LO6=S_LO6, S_HI6=S_HI6, C=C, C6=C6)


def _build(sz):
    S_LO, S_HI, S_LO6, S_HI6, C, C6 = (
        sz["S_LO"], sz["S_HI"], sz["S_LO6"], sz["S_HI6"], sz["C"], sz["C6"]
    )

    nc = bacc.Bacc(num_devices=NCORES, num_swdge_queues=4)
    xT_d = nc.dram_tensor("xTb", [D, B], BF16, kind="ExternalInput")
    il_d = nc.dram_tensor("idx_lo", [128, FULL_SLOTS, S_LO // 16], I16, kind="ExternalInput")
    ih_d = nc.dram_tensor("idx_hi", [128, FULL_SLOTS, S_HI // 16], I16, kind="ExternalInput")
    il6_d = nc.dram_tensor("idx_lo6", [128, S_LO6 // 16], I16, kind="ExternalInput")
    ih6_d = nc.dram_tensor("idx_hi6", [128, S_HI6 // 16], I16, kind="ExternalInput")
    wp_d = nc.dram_tensor("Wp", [FULL_SLOTS, 128, C * O], BF16, kind="ExternalInput")
    wp6_d = nc.dram_tensor("Wp6", [128, C6 * O], BF16, kind="ExternalInput")
    w3_d = nc.dram_tensor("W3l", [128, K2 * E], BF16, kind="ExternalInput")
    bias_d = nc.dram_tensor("bias", [128, K2], F32, kind="ExternalInput")
    b3_d = nc.dram_tensor("b3bc", [16, E], F32, kind="ExternalInput")
    pmat_d = nc.dram_tensor("pmat", [128, 16], BF16, kind="ExternalInput")
    out_d = nc.dram_tensor("out", [16, 2, E], F32, kind="ExternalOutput")

    with tile.TileContext(nc) as tc:
        with (
            tc.tile_pool(name="const", bufs=1) as constp,
            tc.tile_pool(name="gpool", bufs=6) as gpool,
            tc.tile_pool(name="wpool", bufs=6) as wpool,
            tc.tile_pool(name="ps1", bufs=4, space="PSUM") as ps1,
            tc.tile_pool(name="ps2", bufs=1, space="PSUM") as ps2,
            tc.tile_pool(name="psr", bufs=1, space="PSUM") as psr,
            tc.tile_pool(name="dram", bufs=1, space="DRAM") as dramp,
        ):
            il_t = constp.tile([128, FULL_SLOTS, S_LO // 16], I16)
            ih_t = constp.tile([128, FULL_SLOTS, S_HI // 16], I16)
            il6_t = constp.tile([128, S_LO6 // 16], I16)
            ih6_t = constp.tile([128, S_HI6 // 16], I16)
            bias_t = constp.tile([128, K2], F32)
            b3_t = constp.tile([16, E], F32)
            w3_t = constp.tile([128, K2, E], BF16)
            # idx loads ride the second HWDGE ring (Activation) so they are
            # not queued behind the weight streams on sync, and the first
            # gathers can start as early as possible
            nc.scalar.dma_start(il_t[:], il_d[:])
            nc.scalar.dma_start(ih_t[:], ih_d[:])
            nc.scalar.dma_start(il6_t[:], il6_d[:])
            nc.scalar.dma_start(ih6_t[:], ih6_d[:])

            # SDMA round-robins rings at packet granularity, so MB-scale
            # weight streams starve the tiny idx transfers and push the first
            # gather out by ~20us. Gate the big weight DMAs on idx arrival:
            # flag_t reads the idx tiles (RAW dep), and a dummy write into
            # each weight tile (WAW dep) holds its DMA back until then.
            flag_t = constp.tile([1, 16], I16)
            nc.vector.tensor_copy(flag_t[:], il_t[0:1, 0, 0:16])
            nc.vector.tensor_copy(flag_t[:], ih_t[0:1, 0, 0:16])
            nc.vector.tensor_copy(flag_t[:], il6_t[0:1, 0:16])
            nc.vector.tensor_copy(flag_t[:], ih6_t[0:1, 0:16])

            hT_t = constp.tile([128, K2, B], BF16)

            # Gathers spread over the 4 SWDGE queues. Queue 0's desc-gen runs
            # inline on the GpSimd engine (blocks ~10.7us per full gather);
            # queues 1-3 dispatch in ~60ns to async Q7 workers that crunch
            # concurrently. Emission order + queue choice below gets slot 0
            # onto workers immediately, the small slot 5 early, and paces the
            # rest so slots complete roughly every 10.7us.
            gather_order = [0, SLOTS - 1, 1, 2, 3, 4]
            # PE consumption order = order of data readiness
            slot_order = [SLOTS - 1, 0, 1, 2, 3, 4]
            queue_for = {
                (0, 0): 1, (0, 1): 2,
                (SLOTS - 1, 0): 3, (SLOTS - 1, 1): 3,
                (1, 0): 0, (1, 1): 1,
                (2, 0): 2, (2, 1): 3,
                (3, 0): 0, (3, 1): 1,
                (4, 0): 2, (4, 1): 3,
            }
            gts = {}
            for s in gather_order:
                cs = C if s < FULL_SLOTS else C6
                gt = gpool.tile([128, cs, B], BF16, tag="gt" if s < FULL_SLOTS else "gt6")
                if s < FULL_SLOTS:
                    nc.gpsimd.dma_gather(
                        gt[:, 0 : S_LO // 128, :], xT_d[0:HALF, :], il_t[:, s, :],
                        S_LO, S_LO, B, single_packet=False, queue_num=queue_for[(s, 0)],
                    )
                    nc.gpsimd.dma_gather(
                        gt[:, S_LO // 128 : cs, :], xT_d[HALF:D, :], ih_t[:, s, :],
                        S_HI, S_HI, B, single_packet=False, queue_num=queue_for[(s, 1)],
                    )
                else:
                    nc.gpsimd.dma_gather(
                        gt[:, 0 : S_LO6 // 128, :], xT_d[0:HALF, :], il6_t[:],
                        S_LO6, S_LO6, B, single_packet=False, queue_num=queue_for[(s, 0)],
                    )
                    nc.gpsimd.dma_gather(
                        gt[:, S_LO6 // 128 : cs, :], xT_d[HALF:D, :], ih6_t[:],
                        S_HI6, S_HI6, B, single_packet=False, queue_num=queue_for[(s, 1)],
                    )
                wt = wpool.tile([128, cs, O], BF16, tag="wt" if s < FULL_SLOTS else "wt6")
                gts[s] = (gt, wt, cs)

            # Weight streams, sync ring (FIFO). The first ~3MB (s0+s5 wt, W3,
            # small consts) go UNGATED — they drain in the otherwise-dead SDMA
            # window before the first gather descriptors arrive. The remaining
            # 4.7MB (s1-s4 wt) are gated behind idx arrival via flag_t so they
            # cannot starve the tiny idx DMAs at startup.
            def emit_wt_dma(s, gated):
                gt, wt, cs = gts[s]
                if gated:
                    nc.vector.tensor_copy(wt[0:1, 0, 0:16], flag_t[:])
                if s < FULL_SLOTS:
                    nc.sync.dma_start(wt[:], wp_d[s].rearrange("p (c o) -> p c o", o=O))
                else:
                    nc.sync.dma_start(wt[:], wp6_d[:].rearrange("p (c o) -> p c o", o=O))

            emit_wt_dma(0, gated=False)
            emit_wt_dma(SLOTS - 1, gated=False)
            nc.sync.dma_start(w3_t[:], w3_d[:].rearrange("p (k e) -> p k e", e=E))
            nc.sync.dma_start(bias_t[:], bias_d[:])
            nc.sync.dma_start(b3_t[:], b3_d[:])
            pmat_t = constp.tile([128, 16], BF16)
            nc.sync.dma_start(pmat_t[:], pmat_d[:])
            for s in (1, 2, 3, 4):
                emit_wt_dma(s, gated=True)

            # Phase-2 runs in TWO accumulation groups over the same PSUM
            # banks: group A = first 4 consumed slots, group B = last 2.
            # Group A's partial is exchanged with an early AllToAll that
            # overlaps the compute of the last two slots, so only group B's
            # small exchange sits on the critical path after the last GEMM.
            p2_0 = ps2.tile([128, E], F32, tag="p2_0")
            p2_1 = ps2.tile([128, E], F32, tag="p2_1")
            p2 = [p2_0, p2_1]
            GROUP_A_END = 3  # si of the last group-A slot

            def emit_phase2(si, s):
                for bh in range(2):
                    for oh in range(2):
                        kc = s * 2 + oh
                        nc.tensor.matmul(
                            p2[bh][:],
                            hT_t[:, kc, bh * 128 : (bh + 1) * 128],
                            w3_t[:, kc, :],
                            start=(si in (0, GROUP_A_END + 1) and oh == 0),
                            stop=(
                                si in (GROUP_A_END, len(slot_order) - 1)
                                and oh == 1
                            ),
                        )

            part_a = constp.tile([128, 2, E], BF16)
            part_b = constp.tile([128, 2, E], BF16)
            ccin_a = dramp.tile([128, 2, E], BF16)
            ccout_a = dramp.tile([128, 2, E], BF16)
            ccin_b = dramp.tile([128, 2, E], BF16)
            ccout_b = dramp.tile([128, 2, E], BF16)

            def emit_exchange(part, ccin, ccout):
                for bh in range(2):
                    nc.vector.tensor_copy(part[:, bh, :], p2[bh][:])
                nc.sync.dma_start(ccin[:], part[:])
                nc.gpsimd.collective_compute(
                    "AllToAll",
                    mybir.AluOpType.bypass,
                    replica_groups=[list(range(NCORES))],
                    ins=[ccin[:].opt()],
                    outs=[ccout[:].opt()],
                )

            # phase-2 for slot k is emitted during slot k+1's phase-1 so the
            # PE never waits on the DVE bias-add round trip
            for si, s in enumerate(slot_order):
                gt, wt, cs = gts[s]
                for oh in range(2):
                    ps = ps1.tile([128, B], F32)
                    for cc in range(cs):
                        nc.tensor.matmul(
                            ps[:],
                            wt[:, cc, oh * 128 : (oh + 1) * 128],
                            gt[:, cc, :],
                            start=(cc == 0),
                            stop=(cc == cs - 1),
                        )
                    kc = s * 2 + oh
                    nc.vector.tensor_scalar_add(
                        hT_t[:, kc, :], ps[:], bias_t[:, kc : kc + 1]
                    )
                if si > 0:
                    emit_phase2(si - 1, slot_order[si - 1])
                    if si - 1 == GROUP_A_END:
                        emit_exchange(part_a, ccin_a, ccout_a)
            emit_phase2(len(slot_order) - 1, slot_order[-1])
            emit_exchange(part_b, ccin_b, ccout_b)

            # pmat (ones at (i, i%16)) matmul sums the 8 per-core
            # contributions that land on partition groups 16j..16j+16 — a
            # cross-partition reduce the DVE cannot do; both exchanges
            # accumulate into the same PSUM bank. bf16 partials on the wire
            # add ~0.1% error (budget 2e-2).
            a2a_ta = constp.tile([128, 2, E], BF16)
            nc.sync.dma_start(a2a_ta[:], ccout_a[:])
            a2a_tb = constp.tile([128, 2, E], BF16)
            nc.sync.dma_start(a2a_tb[:], ccout_b[:])
            pr = [
                psr.tile([16, E], F32, tag=f"pr{bh}", name=f"pr{bh}")
                for bh in range(2)
            ]
            for bh in range(2):
                nc.tensor.matmul(
                    pr[bh][:], pmat_t[:], a2a_ta[:, bh, :], start=True, stop=False,
                )
                nc.tensor.matmul(
                    pr[bh][:], pmat_t[:], a2a_tb[:, bh, :], start=False, stop=True,
                )
            z_t = constp.tile([16, 2, E], F32)
            for bh in range(2):
                nc.vector.tensor_add(z_t[:, bh, :], pr[bh][:], b3_t[:])
            o_t = constp.tile([16, 2, E], F32)
            # LeakyReLU: max(0.2*z, z)
            nc.vector.scalar_tensor_tensor(
                o_t[:], z_t[:], NEG_SLOPE, z_t[:],
                op0=mybir.AluOpType.mult, op1=mybir.AluOpType.max,
            )
            nc.sync.dma_start(out_d[:], o_t[:])
    nc.compile()
    return nc


def kernel_with_results(x, group_idx, W, b, W3, b3, trace=False, warmup=True):
    in_maps, sz = _prep_inputs(
        np.asarray(x, dtype=np.float32),
        np.asarray(group_idx),
        np.asarray(W, dtype=np.float32),
        np.asarray(b, dtype=np.float32),
        np.asarray(W3, dtype=np.float32),
        np.asarray(b3, dtype=np.float32),
    )
    nc = _build(sz)
    if warmup:
        # first execute pays NEFF-load / runtime-init cross-core skew; the
        # measured run below then starts with all 8 cores aligned
        run_bass_kernel_spmd(nc, in_maps, core_ids=list(range(NCORES)))
    import os as _os

    extra = int(_os.environ.get("N_RUNS", "1")) - 1
    for _ in range(extra):
        r = run_bass_kernel_spmd(
            nc, in_maps, core_ids=list(range(NCORES)), trace=trace
        )
        if r.exec_time_ns is not None:
            print(f"run exec: {r.exec_time_ns} ns")
    res = run_bass_kernel_spmd(
        nc, in_maps, core_ids=list(range(NCORES)), trace=trace
    )
    out = np.empty((B, E), np.float32)
    for c in range(NCORES):
        shard = res.results[c]["out"]  # (16, 2, E): rows 16c..16c+16 of each b-half
        out[16 * c : 16 * c + 16, :] = shard[:, 0, :]
        out[128 + 16 * c : 128 + 16 * c + 16, :] = shard[:, 1, :]
    return out, res


def kernel(**inputs):
    out, _ = kernel_with_results(**inputs)
    return out

